# revision 14
# baseline (speedup 1.0000x reference)
"""DBLoss (OHEM-masked BCE + masked L1 threshold loss) on 8 Trainium2 cores.

Shapes are hardcoded for the nn_DBLoss problem:
  outputs             [16, 3, 640, 640] f32
  gt_shrink_labels    [16, 640, 640]    f32
  gt_threshold_labels [16, 640, 640]    f32
Returns np.float32[4] = (loss_all, loss_shrink, loss_binary, loss_thresh).

Sharding: pure data parallel - 2 images per core, 8 cores. Each core computes
per-image partial sums (per-partition [128] vectors); the host reduces the
tiny partials and forms the masked means.

v5 design (vs the exp/ln-chain baseline):
 * Host stages the three logits planes (shrink, threshold, binary) as
   fp8e4 and the two label planes as fp16 (memory regime: 2.9x fewer HBM
   bytes than f32). The losses are means over 409600 pixels: fp8 logit
   rounding is unbiased and second-order in the loss (measured end-to-end
   rel err ~1e-4, tolerance 2e-2).
 * ACT (the bottleneck engine, 1 elem/cycle/lane at 1.2 GHz regardless of
   dtype): native Sigmoid table for the two tm planes (this platform's PWP
   has no softplus table), then one switch to natural_log_exp_and_others
   for the four BCE softplus sums as exp then ln(1+u)-accumulate: 10 table
   passes + 1 on-path table load. A dummy sigmoid at t=0 pulls the first
   table load off the critical path; sigmoids also emit sum(sig) through
   accum_out for free.
 * DVE: six fused scalar_tensor_tensor passes with f32 accumulators
   (InstTensorScalarPtr runs 1x, ~3.4us/pass): four masked sums
   (g > 0.5) * x and two mins for the L1 term via
   sum|sig-gt| = sum(sig) + sum(gt) - 2*sum(min(sig,gt)),
   where sum(gt) is an exact host-side sum (same spirit as the host-side
   pos_num regime check).
 * The partials DMA out in two slices (DVE columns as soon as DVE drains,
   ACT BCE columns at the end).
 * Same fast-path math as the baseline: with neg_num == neg_total the OHEM
   mask is all-ones for every valid image (host verifies, exact numpy
   fallback otherwise); BCE reduces to softplus(x) - t*x; threshold-loss mask
   corrections for (gt_t<=0)&(gt_s<=0) pixels are applied on the host.
"""

import sys

import numpy as np

try:
    import concourse.bass as bass
except ImportError:  # stand-alone grading dir: fall back to known repo paths
    for _p in ("/root/.axon_site/_ro/trn_rl_repo", "/opt/trn_rl_repo"):
        if _p not in sys.path:
            sys.path.append(_p)
    import concourse.bass as bass

from concourse import mybir
from concourse.bass_utils import run_bass_kernel_spmd

B, H, W = 16, 640, 640
N = H * W                    # 409600 pixels / image
P = 128                      # SBUF partitions
F = N // P                   # 3200 free elements / partition
NCORES = 8
BPC = B // NCORES            # 2 images per core
ALPHA, BETA = 1.0, 10.0
F16 = mybir.dt.float16
F32 = mybir.dt.float32
F8 = mybir.dt.float8e4

# fp8 plane order within data8: tm0, tm1, s0, bn0, s1, bn1
P8_TM0, P8_TM1, P8_S0, P8_BN0, P8_S1, P8_BN1 = range(6)
# fp16 plane order within data16: g0, gt0, g1, gt1
P16_G0, P16_GT0, P16_G1, P16_GT1 = range(4)

_CACHED_NC = None


def build_nc() -> "bass.Bass":
    """Per-core raw-bass program.

    po [128, 12] f32 accumulator columns:
      0: sum softplus(s0)   1: sum softplus(bn0)
      2: sum softplus(s1)   3: sum softplus(bn1)
      4: sum t0*s0  5: sum t0*bn0  6: sum t1*s1  7: sum t1*bn1
      8: sum min(sig0,gt0)  9: sum min(sig1,gt1)
     10: sum sig0          11: sum sig1

    Raw bass (no TileContext): cross-engine ordering uses standalone wait_ge
    instructions with explicit semaphores; every data op carries then_inc on
    its own engine counter (fires on write-ack, so waiting on the counter
    also guards RAW across engines).
    """
    nc = bass.Bass(dynamic_dma_scratch_size=2048, enable_partition_id=False,
                   monotonic_sem_count=0)
    data8 = nc.dram_tensor("data8", [6, P, F], F8, kind="ExternalInput")
    data16 = nc.dram_tensor("data16", [4, P, F], F16, kind="ExternalInput")
    part = nc.dram_tensor("part", [P, 12], F32, kind="ExternalOutput")

    EXP = mybir.ActivationFunctionType.Exp
    LN = mybir.ActivationFunctionType.Ln
    SG = mybir.ActivationFunctionType.Sigmoid
    ag = mybir.AluOpType.is_gt
    mul = mybir.AluOpType.mult
    mn = mybir.AluOpType.min

    from contextlib import ExitStack
    ctx = ExitStack()
    with ctx:
        sb = lambda nm, shape, dt: ctx.enter_context(nc.sbuf_tensor(nm, shape, dt))
        sem = lambda nm: ctx.enter_context(nc.semaphore(name=nm))
        T8 = [sb(f"t8_{i}", [P, F], F8) for i in range(6)]
        T16 = [sb(f"t16_{i}", [P, F], F16) for i in range(4)]
        SGo = [sb("sg0", [P, F], F16), sb("sg1", [P, F], F16)]
        E = [sb("e0", [P, F], F32), sb("e1", [P, F], F32),
             sb("e2", [P, F], F32)]
        pr2 = sb("pr2", [P, F // 2], F32)
        pr4 = sb("pr4", [P, F // 4], F32)
        w = sb("w", [P, F], F16)
        dum = sb("dum", [P, 1], F16)
        po = sb("po", [P, 12], F32)
        # DMA-completion semaphores in ISSUE order:
        #  0: tm0  1: tm1  2: s0  3: g0  4: bn0  5: gt0  6: s1  7: g1
        #  8: bn1  9: gt1
        d = [sem(f"d{i}") for i in range(10)]
        sa, sv, dout = sem("sa"), sem("sv"), sem("dout")
        block = ctx.enter_context(nc.Block(no_gpsimd_drain=True))

        issue = [
            (T8[P8_TM0], data8[P8_TM0]), (T8[P8_TM1], data8[P8_TM1]),
            (T8[P8_S0], data8[P8_S0]), (T16[P16_G0], data16[P16_G0]),
            (T8[P8_BN0], data8[P8_BN0]), (T16[P16_GT0], data16[P16_GT0]),
            (T8[P8_S1], data8[P8_S1]), (T16[P16_G1], data16[P16_G1]),
            (T8[P8_BN1], data8[P8_BN1]), (T16[P16_GT1], data16[P16_GT1]),
        ]
        D_TM0, D_TM1, D_S0, D_G0, D_BN0, D_GT0, D_S1, D_G1, D_BN1, D_GT1 = range(10)

        @block.sync
        def _(sync):
            # tm0 gates the whole ACT chain: ship its first half here and
            # its second half on the scalar-issued HWDGE ring (below) so the
            # two halves transfer concurrently; d[0] completes at >= 32.
            h = F // 2
            sync.dma_start(
                out=T8[P8_TM0][:, :h], in_=data8[P8_TM0][:, :h]
            ).then_inc(d[0], 16)
            for i, (dst, src) in enumerate(issue):
                if i == 0:
                    continue
                sync.dma_start(out=dst[:, :], in_=src).then_inc(d[i], 16)
            # DVE columns (4..11) ship as soon as DVE + the sigmoids drain
            sync.wait_ge(sa, 2)
            sync.wait_ge(sv, 9)
            sync.dma_start(out=part[:, 4:12], in_=po[:, 4:12]).then_inc(dout, 16)
            # ACT BCE columns (0..4) ship at the end
            sync.wait_ge(sa, 10)
            sync.dma_start(out=part[:, 0:4], in_=po[:, 0:4]).then_inc(dout, 16)
            for s_ in d + [sa, sv]:
                sync.sem_clear(s_)
            sync.wait_ge(dout, 32)
            sync.sem_clear(dout)

        @block.scalar
        def _(scalar):
            # second half of tm0 on the ACT-issued HWDGE ring (concurrent
            # with the sync ring) - fire-and-forget, ~50ns of engine time
            h = F // 2
            scalar.dma_start(
                out=T8[P8_TM0][:, h:], in_=data8[P8_TM0][:, h:]
            ).then_inc(d[0], 16)
            # Dummy 1-column sigmoid: walrus places the sigmoid-set
            # ACT_TABLE_LOAD before it, so the load overlaps the first DMA.
            nc.scalar.activation(out=dum[:, 0:1], in_=dum[:, 0:1], func=SG)
            # sigmoids (fp8 in, fp16 out) with free sum(sig) accums
            for k, (ti, di) in enumerate(((P8_TM0, D_TM0), (P8_TM1, D_TM1))):
                scalar.wait_ge(d[di], 32 if di == D_TM0 else 16)
                nc.scalar.activation(
                    out=SGo[k][:, :], in_=T8[ti][:, :], func=SG,
                    accum_out=po[:, 10 + k : 11 + k],
                ).then_inc(sa, 1)
            # table switch to natural_log_exp happens before the first
            # Exp. Channel s0's ln is offloaded: DVE forms exact pairwise
            # products p4 = prod(1+e^x) over groups of 4, ACT finishes with a
            # quarter-size ln(p4) accumulate (sum ln(1+e^x) = sum ln p4).
            scalar.wait_ge(d[D_S0], 16)
            nc.scalar.activation(
                out=E[0][:, :], in_=T8[P8_S0][:, :], func=EXP,
            ).then_inc(sa, 1)
            for ti, di, col, e in ((P8_BN0, D_BN0, 1, E[1]),
                                   (P8_S1, D_S1, 2, E[2]),
                                   (P8_BN1, D_BN1, 3, E[1])):
                scalar.wait_ge(d[di], 16)
                nc.scalar.activation(
                    out=e[:, :], in_=T8[ti][:, :], func=EXP,
                ).then_inc(sa, 1)
                nc.scalar.activation(
                    out=e[:, :], in_=e[:, :], func=LN, bias=1.0,
                    accum_out=po[:, col : col + 1],
                ).then_inc(sa, 1)
            scalar.wait_ge(sv, 5)
            nc.scalar.activation(
                out=pr4[:, :], in_=pr4[:, :], func=LN,
                accum_out=po[:, 0:1],
            ).then_inc(sa, 1)

        @block.vector
        def _(vector):
            def stt(t0, t1, op0, op1, col, scalar=1.0):
                nc.vector.scalar_tensor_tensor(
                    out=w[:, :], in0=t0[:, :], scalar=scalar,
                    in1=t1[:, :], op0=op0, op1=op1,
                    accum_out=po[:, col : col + 1],
                ).then_inc(sv, 1)

            # masked sums image 0: (g0 > 0.5) * s0 / bn0
            vector.wait_ge(d[D_G0], 16)
            vector.wait_ge(d[D_S0], 16)
            stt(T16[P16_G0], T8[P8_S0], ag, mul, 4, scalar=0.5)
            vector.wait_ge(d[D_BN0], 16)
            stt(T16[P16_G0], T8[P8_BN0], ag, mul, 5, scalar=0.5)
            # s0 ln-offload products: E0 <- 1 + e^x in place, then two exact
            # pairwise halvings (f32; max (1+e^16)^4 ~ 6e27 stays finite)
            vector.wait_ge(sa, 3)
            nc.vector.tensor_scalar(
                out=E[0][:, :], in0=E[0][:, :], scalar1=1.0, scalar2=None,
                op0=mybir.AluOpType.add,
            ).then_inc(sv, 1)
            nc.vector.tensor_tensor(
                out=pr2[:, :], in0=E[0][:, 0 : F : 2], in1=E[0][:, 1 : F : 2],
                op=mul,
            ).then_inc(sv, 1)
            nc.vector.tensor_tensor(
                out=pr4[:, :], in0=pr2[:, 0 : F // 2 : 2],
                in1=pr2[:, 1 : F // 2 : 2], op=mul,
            ).then_inc(sv, 1)
            # L1 image 0: sum min(sig0, gt0)
            vector.wait_ge(sa, 1)
            vector.wait_ge(d[D_GT0], 16)
            stt(SGo[0], T16[P16_GT0], mul, mn, 8)
            # masked sums image 1
            vector.wait_ge(d[D_G1], 16)
            vector.wait_ge(d[D_S1], 16)
            stt(T16[P16_G1], T8[P8_S1], ag, mul, 6, scalar=0.5)
            vector.wait_ge(d[D_BN1], 16)
            stt(T16[P16_G1], T8[P8_BN1], ag, mul, 7, scalar=0.5)
            # L1 image 1
            vector.wait_ge(sa, 2)
            vector.wait_ge(d[D_GT1], 16)
            stt(SGo[1], T16[P16_GT1], mul, mn, 9)

    return nc


def _numpy_reference(outputs, gt_shrink_labels, gt_threshold_labels):
    """Exact fallback for inputs outside the fast-path regime."""
    OHEM_RATIO, EPS = 3, 1e-7

    def sigmoid(x):
        return 1.0 / (1.0 + np.exp(-x))

    shrink, thresh, binary = outputs[:, 0], outputs[:, 1], outputs[:, 2]
    b = outputs.shape[0]
    flat_s = shrink.reshape(b, -1)
    flat_pos = (gt_shrink_labels > 0.5).reshape(b, -1)
    n = flat_s.shape[1]
    pos_num = flat_pos.sum(axis=1)
    neg_total = n - pos_num
    neg_num = np.minimum(pos_num * OHEM_RATIO, neg_total)
    neg_scores = np.where(flat_pos, -np.inf, flat_s)
    sorted_desc = -np.sort(-neg_scores, axis=1)
    idx = np.clip(neg_num - 1, 0, n - 1).astype(np.int64)
    thr = np.take_along_axis(sorted_desc, idx[:, None], axis=1)
    mask = (flat_s >= thr) | flat_pos
    valid = (pos_num > 0) & (neg_num > 0)
    mask = (mask & valid[:, None]).reshape(shrink.shape).astype(np.float32)

    def masked_bce(logits, target, m):
        p = np.clip(sigmoid(logits), EPS, 1.0 - EPS)
        t = (target > 0.5).astype(np.float32)
        per_px = -(t * np.log(p) + (1.0 - t) * np.log(1.0 - p))
        denom = m.sum()
        return float(per_px.flatten() @ m.flatten() / max(denom, 1.0)) if denom > 0 else 0.0

    loss_shrink = masked_bce(shrink, gt_shrink_labels, mask)
    loss_binary = masked_bce(binary, gt_shrink_labels, mask)
    m2 = ((gt_threshold_labels > 0) | (gt_shrink_labels > 0)).astype(np.float32)
    denom2 = m2.sum()
    l1 = np.abs(sigmoid(thresh) - gt_threshold_labels).flatten() @ m2.flatten()
    loss_thresh = float(l1 / max(denom2, 1.0)) if denom2 > 0 else 0.0
    loss_all = loss_shrink + ALPHA * loss_binary + BETA * loss_thresh
    return np.array([loss_all, loss_shrink, loss_binary, loss_thresh], np.float32)


def kernel(outputs, gt_shrink_labels, gt_threshold_labels, _trace=False):
    global _CACHED_NC
    outputs = np.ascontiguousarray(np.asarray(outputs, dtype=np.float32))
    gts = np.ascontiguousarray(np.asarray(gt_shrink_labels, dtype=np.float32))
    gtt = np.ascontiguousarray(np.asarray(gt_threshold_labels, dtype=np.float32))

    # ---- host-side regime checks (exactness guards for the fast path) ----
    pos_num = (gts > 0.5).reshape(B, -1).sum(axis=1)
    neg_total = N - pos_num
    neg_num = np.minimum(3 * pos_num, neg_total)
    valid = (pos_num > 0) & (neg_num > 0)
    needs_topk = valid & (3 * pos_num < neg_total)
    clip_active = max(
        float(np.abs(outputs[:, 0]).max()), float(np.abs(outputs[:, 2]).max())
    ) >= 16.0
    if needs_topk.any() or clip_active:
        return _numpy_reference(outputs, gts, gtt)

    if _CACHED_NC is None:
        _CACHED_NC = build_nc()
    nc = _CACHED_NC

    # ---- staging: logits fp8e4, labels fp16, packed per core ----
    f8np = mybir.dt.np(F8)
    big8 = np.empty((NCORES, 6, P, F), f8np)
    big16 = np.empty((NCORES, 4, P, F), np.float16)
    for c in range(NCORES):
        i0, i1 = c * BPC, c * BPC + 1
        big8[c, P8_TM0] = outputs[i0, 1].reshape(P, F).astype(f8np)
        big8[c, P8_TM1] = outputs[i1, 1].reshape(P, F).astype(f8np)
        big8[c, P8_S0] = outputs[i0, 0].reshape(P, F).astype(f8np)
        big8[c, P8_BN0] = outputs[i0, 2].reshape(P, F).astype(f8np)
        big8[c, P8_S1] = outputs[i1, 0].reshape(P, F).astype(f8np)
        big8[c, P8_BN1] = outputs[i1, 2].reshape(P, F).astype(f8np)
        big16[c, P16_G0] = gts[i0].reshape(P, F)
        big16[c, P16_GT0] = gtt[i0].reshape(P, F)
        big16[c, P16_G1] = gts[i1].reshape(P, F)
        big16[c, P16_GT1] = gtt[i1].reshape(P, F)

    in_maps = [{"data8": big8[c], "data16": big16[c]} for c in range(NCORES)]
    res = run_bass_kernel_spmd(
        nc, in_maps, core_ids=list(range(NCORES)), trace=_trace
    )

    # ---- host combine: per-image sums from per-partition partials ----
    sp_s = np.empty(B); sp_b = np.empty(B); ts = np.empty(B); tb = np.empty(B)
    l1 = np.empty(B)
    # exact per-image sums of the staged fp16 gt_threshold planes
    sum_gt = big16[:, (P16_GT0, P16_GT1)].astype(np.float64).sum(axis=(2, 3))
    for c in range(NCORES):
        po = res.results[c]["part"].astype(np.float64).sum(axis=0)
        i0, i1 = c * BPC, c * BPC + 1
        sp_s[i0], sp_b[i0] = po[0], po[1]
        sp_s[i1], sp_b[i1] = po[2], po[3]
        ts[i0], tb[i0] = po[4], po[5]
        ts[i1], tb[i1] = po[6], po[7]
        l1[i0] = po[10] + sum_gt[c, 0] - 2.0 * po[8]
        l1[i1] = po[11] + sum_gt[c, 1] - 2.0 * po[9]

    cnt = float(N * valid.sum())
    num_s = float(((sp_s - ts) * valid).sum())
    num_b = float(((sp_b - tb) * valid).sum())
    loss_shrink = num_s / max(cnt, 1.0) if cnt > 0 else 0.0
    loss_binary = num_b / max(cnt, 1.0) if cnt > 0 else 0.0

    # threshold-loss mask corrections for pixels where both labels <= 0
    zz = (gtt <= 0) & (gts <= 0)
    cnt2 = float(B * N - zz.sum())
    l1_tot = float(l1.sum())
    if zz.any():
        tmz = outputs[:, 1][zz]
        l1_tot -= float(np.abs(1.0 / (1.0 + np.exp(-tmz)) - gtt[zz]).sum())
    loss_thresh = l1_tot / max(cnt2, 1.0) if cnt2 > 0 else 0.0

    loss_all = loss_shrink + ALPHA * loss_binary + BETA * loss_thresh
    out = np.array([loss_all, loss_shrink, loss_binary, loss_thresh], np.float32)
    if _trace:
        return out, res
    return out
